# revision 5
# baseline (speedup 1.0000x reference)
"""Trainium2 Bass kernel for the binary-conv BasicBlock (dense_cnn).

Computation (forward values only):
  A1   = sign(x + b11)
  out1 = x + bn1(conv3x3(A1, binw(w3)))          binw(w) = mean|w| * sign(w)
  o1   = prelu(out1 + b12, a1) + b13
  A2   = sign(o1 + b21)
  out2 = bn2(conv1x1(A2, binw(w1))) + o1
  out  = prelu(out2 + b22, a2) + b23

Strategy: data-parallel over the batch axis, 4 images per NeuronCore on 8
cores; weights/consts replicated.  Per core the 3x3 binary conv runs as 9
shifted DoubleRow fp8 matmuls over 464-column tiles spanning padded rows
1..56 only (3248 cols).  Element-wise work is spread over all three
engines so the TensorE (~27.6us/image) is the only bottleneck:
  DVE:    (a) t = psum1*sh1 + xprep      (d) u = psum2*sh2 + p1   [in-place]
  ACT:    prep sign(A1), (b) p1 = prelu(t, a1), (e) prelu(u + K2', a2)
  GpSimd: (c) a2' = 2*[t >= tau] in {0,2} fp8, (g) + b23
conv2 uses the exact identity conv(W, 2B-1) = conv(W, 2B) - rowsum(W),
with rowsum folded into the (e) bias, so binarization is a single
tensor_scalar op instead of an ACT sign.  a2' = 2*[t >= tau] replaces
sign(prelu(t,a1)+beta) exactly for a1 > 0 (tau = -beta/a1 if beta >= 0
else -beta).  conv2 matmuls of image i are interleaved into image i+1's
conv1 PE stream via a pending queue so the PE never idles on drains.
Image 0's A1 is built in 4 row-bands so the first matmul starts ~3us in.
"""

import numpy as np
import ml_dtypes
from collections import deque

C = 256
H = W = 56
PH = 58                    # padded image side
NPIX = PH * PH             # 3364
TW = 464                   # matmul tile width (8 padded rows)
NT = 7                     # tiles per image
TOT = NT * TW              # 3248 = 56 rows * 58 cols  (padded rows 1..56)
G0 = PH                    # first computed pixel (row 1, col 0) in A1 coords
HALO = 16
A1W = 3408                 # 16 + NPIX + 16 -> next multiple of 16
BPC = 4                    # images per core
NCORES = 8
EPS = 1e-5

# img0 band layout: (first padded row, end padded row, MM tiles in band)
BANDS = [(0, 19, (0, 1)), (16, 35, (2, 3)), (32, 51, (4, 5)), (48, 58, (6,))]
TILE2BAND = [0, 0, 1, 1, 2, 2, 3]


def _pad16(w):
    return (w + 15) // 16 * 16


_CACHE = {}
_FLAGS = {"has_b23": False}


def _split_drain_waits(m, max_waits=1):
    """This toolchain's walrus rejects instructions carrying more than ~1-2
    sync waits; hoist extra waits onto preceding single-wait EventSemaphore
    ops on the same engine (semantically identical: the engine blocks on
    each wait in sequence before executing the instruction)."""
    import copy as _copy
    from concourse import mybir

    new_module = _copy.replace(m, functions=[])
    for function in m.functions:
        new_function = _copy.replace(function, blocks=[])
        new_function.set_allocations_from_list(function.allocations)
        for block in function.blocks:
            out = []
            for inst in block.instructions:
                si = inst.sync_info
                if si is not None and len(si.on_wait) > max_waits:
                    waits = list(si.on_wait)
                    keep = waits[:max_waits] if not isinstance(
                        inst, mybir.InstDrain) else []
                    hoist = waits[len(keep):]
                    for i, wt in enumerate(hoist):
                        out.append(
                            mybir.InstEventSemaphore(
                                name=f"{inst.name}-wsplit{i}",
                                opcode="EventSemaphore",
                                engine=inst.engine,
                                sync_info=mybir.SyncInfo(on_wait=[wt], on_update=[]),
                            )
                        )
                    inst.sync_info = mybir.SyncInfo(
                        on_wait=keep, on_update=list(si.on_update)
                    )
                out.append(inst)
            new_block = _copy.replace(block, instructions=out)
            new_function.blocks.append(new_block)
        new_module.functions.append(new_function)
    return new_module


def build_nc(has_b23=None):
    """Build (once per structure flag) the per-core Bass program."""
    if has_b23 is None:
        has_b23 = _FLAGS["has_b23"]
    key = ("nc", has_b23)
    if key in _CACHE:
        return _CACHE[key]
    import concourse.bass as bass
    import concourse.tile as tile
    from concourse import mybir

    Alu = mybir.AluOpType
    AF = mybir.ActivationFunctionType
    f32 = mybir.dt.float32
    bf16 = mybir.dt.bfloat16
    fp8 = mybir.dt.float8e4
    DR = mybir.MatmulPerfMode.DoubleRow

    nc = bass.Bass(trn_type="TRN2", debug=False)
    # xprep = x + K1, stored for padded rows 1..56 (58 cols each, col 0/57
    # are zero pads): dram index = (row-1)*58 + col
    x_d = nc.dram_tensor("xprep", [BPC, 2, 128, TOT], f32, kind="ExternalInput")
    w3_d = nc.dram_tensor("w3f", [128, 9 * 2 * 2 * 128], fp8, kind="ExternalInput")
    w1_d = nc.dram_tensor("w1f", [128, 2 * 2 * 128], fp8, kind="ExternalInput")
    c_d = nc.dram_tensor("consts", [2, 128, 8], f32, kind="ExternalInput")
    o_d = nc.dram_tensor("out", [BPC, 2, 128, H * W], f32, kind="ExternalOutput")

    with tile.TileContext(nc) as tc:
        with (
            tc.tile_pool(name="wpool", bufs=1) as wpool,
            tc.tile_pool(name="xbpool", bufs=1) as xbpool,
            tc.tile_pool(name="a1bpool", bufs=1) as a1bpool,
            tc.tile_pool(name="xpool", bufs=2) as xpool,
            tc.tile_pool(name="a1pool", bufs=2) as a1pool,
            tc.tile_pool(name="tpool", bufs=2) as tpool,
            tc.tile_pool(name="p1pool", bufs=2) as p1pool,
            tc.tile_pool(name="a2pool", bufs=9) as a2pool,
            tc.tile_pool(name="opool", bufs=1) as opool,
            tc.tile_pool(name="ps1", bufs=4, space="PSUM") as ps1p,
            tc.tile_pool(name="ps2", bufs=4, space="PSUM") as ps2p,
        ):
            # ---- constants / weights (resident) ----
            w3sb = wpool.tile([128, 9 * 2 * 2 * 128], fp8, tag="w3")
            nc.sync.dma_start(w3sb[:], w3_d.ap())
            w1sb = wpool.tile([128, 2 * 2 * 128], fp8, tag="w1")
            nc.sync.dma_start(w1sb[:], w1_d.ap())
            w3v = w3sb[:].rearrange("p (g two m) -> p g two m", two=2, m=128)
            w1v = w1sb[:].rearrange("p (g two m) -> p g two m", two=2, m=128)
            csb = []
            for kc in range(2):
                ct = wpool.tile([128, 8], f32, tag=f"c_{kc}")
                nc.sync.dma_start(ct[:], c_d.ap()[kc])
                csb.append(ct)

            def cc(kc, j):
                return csb[kc][:, j : j + 1]

            # const j-layout: 0 bias1 (prep sign), 1 tau, 2 K2', 3 a1,
            # 4 a2, 5 b23, 6 sh1, 7 sh2

            # ---- per-image state ----
            xb = {}    # (band, kc) -> img0 xprep band tile
            a1b = []   # img0 A1 band tiles
            xm = [None] * BPC   # mono xprep tiles (imgs 1..3): [kc]
            a1m = [None] * BPC  # mono A1 tiles (imgs 1..3)
            tbs = [None] * BPC  # t tiles per mc (bf16)
            pbs = [None] * BPC  # p1/u tiles per mc (bf16)
            segs = [None] * BPC  # a2 segment tiles per MM tile
            pending = deque()   # conv2 closures: one MM2 + one (d) each

            def memset_borders(a1t, r0, r1, width):
                # zero every A1 element a matmul may read that sign won't
                # write: halo, row 0 / row 57, and col 0/57 of each row.
                nrow = r1 - r0
                for kc in range(2):
                    v = a1t[:].rearrange("p (two w) -> p two w", two=2)[:, kc]
                    head = HALO + (PH + 1 if r0 == 0 else 1)
                    nc.gpsimd.memset(v[:, 0:head], 0.0)
                    # (row r, c57)+(row r+1, c0) pairs for r in [r0, r1-1)
                    pairs = v[
                        :, HALO + 57 : HALO + 57 + (nrow - 1) * PH
                    ].rearrange("p (h w) -> p h w", w=PH)[:, :, 0:2]
                    nc.gpsimd.memset(pairs, 0.0)
                    if r1 == PH:
                        # row 57 (minus its c0, already in the last pair)
                        # plus right halo / alignment tail
                        nc.gpsimd.memset(
                            v[:, HALO + (57 - r0) * PH + 1 : width], 0.0
                        )
                    else:
                        # last row's c57 (+1 spare into the unread margin)
                        nc.gpsimd.memset(
                            v[:, HALO + (nrow - 1) * PH + 57 :
                              HALO + (nrow - 1) * PH + 59], 0.0
                        )

            def prep_img0():
                for b, (r0, r1, _tiles) in enumerate(BANDS):
                    vr0, vr1 = max(r0, 1), min(r1, 57)
                    for kc in range(2):
                        xt = xbpool.tile(
                            [128, (vr1 - vr0) * PH], f32, tag=f"xb{b}_{kc}",
                            name=f"xb{b}_{kc}",
                        )
                        nc.sync.dma_start(
                            xt[:],
                            x_d.ap()[0, kc][:, (vr0 - 1) * PH : (vr1 - 1) * PH],
                        )
                        xb[(b, kc)] = xt
                for b, (r0, r1, _tiles) in enumerate(BANDS):
                    width = _pad16(HALO + (r1 - r0) * PH + HALO)
                    a1t = a1bpool.tile([128, 2 * width], fp8, tag=f"a1b{b}",
                                       name=f"a1b{b}")
                    a1b.append(a1t)
                    memset_borders(a1t, r0, r1, width)
                    vr0, vr1 = max(r0, 1), min(r1, 57)
                    nvr = vr1 - vr0
                    for kc in range(2):
                        dst = a1t[:].rearrange("p (two w) -> p two w", two=2)[
                            :, kc, HALO + (vr0 - r0) * PH : HALO + (vr1 - r0) * PH
                        ].rearrange("p (h w) -> p h w", w=PH)[:, :, 1:57]
                        src = xb[(b, kc)][:].rearrange(
                            "p (h w) -> p h w", w=PH
                        )[:, :, 1:57]
                        nc.scalar.activation(dst, src, AF.Sign, bias=cc(kc, 0))

            def prep_mono(img):
                # called from hooks of conv1(img-1): t0/t2 DMA, t4/t5 sign
                xm[img] = [
                    xpool.tile([128, TOT], f32, tag=f"xk_{kc}", name=f"xk_{kc}")
                    for kc in range(2)
                ]
                a1m[img] = a1pool.tile([128, 2 * A1W], fp8, tag="a1m", name="a1m")

            def prep_mono_dma(img, kc):
                nc.sync.dma_start(xm[img][kc][:], x_d.ap()[img, kc])

            def prep_mono_sign(img, kc):
                if kc == 0:
                    memset_borders(a1m[img], 0, PH, A1W)
                dst = a1m[img][:].rearrange("p (two w) -> p two w", two=2)[
                    :, kc, HALO + G0 : HALO + G0 + TOT
                ].rearrange("p (h w) -> p h w", w=PH)[:, :, 1:57]
                src = xm[img][kc][:].rearrange("p (h w) -> p h w", w=PH)[:, :, 1:57]
                nc.scalar.activation(dst, src, AF.Sign, bias=cc(kc, 0))

            def a1_rhs(img, t, kh, kw):
                if img == 0:
                    b = TILE2BAND[t]
                    r0 = BANDS[b][0]
                    base = HALO + (G0 + TW * t - r0 * PH)
                    v = a1b[b][:].rearrange("p (two w) -> p two w", two=2)
                else:
                    base = HALO + G0 + TW * t
                    v = a1m[img][:].rearrange("p (two w) -> p two w", two=2)
                off = base + (kh - 1) * PH + (kw - 1)
                return v[:, :, off : off + TW]

            def xprep_slice(img, t, mc):
                if img == 0:
                    b = TILE2BAND[t]
                    vr0 = max(BANDS[b][0], 1)
                    lo = TW * t - (vr0 - 1) * PH
                    return xb[(b, mc)][:, lo : lo + TW]
                return xm[img][mc][:, TW * t : TW * (t + 1)]

            def emit_b(img, lo, hi):
                for mc in range(2):
                    nc.scalar.activation(
                        pbs[img][mc][:, lo:hi], tbs[img][mc][:, lo:hi],
                        AF.Prelu, alpha=cc(mc, 3),
                    )

            def emit_eg(img, h0, h1):
                # (e) prelu(u + K2', a2) interior -> compact out, (g) +b23,
                # then DMA the finished rows
                for mc in range(2):
                    ot = state_out[img][mc]
                    dst = ot[:].rearrange("p (h w) -> p h w", w=W)[:, h0:h1, :]
                    src = pbs[img][mc][:].rearrange(
                        "p (h w) -> p h w", w=PH
                    )[:, h0:h1, 1:57]
                    nc.scalar.activation(
                        dst, src, AF.Prelu, bias=cc(mc, 2), alpha=cc(mc, 4)
                    )
                    if has_b23:
                        nc.gpsimd.tensor_scalar(
                            dst, dst, cc(mc, 5), None, Alu.add
                        )
                    nc.sync.dma_start(
                        o_d.ap()[img, mc][:, h0 * W : h1 * W],
                        ot[:, h0 * W : h1 * W],
                    )

            state_out = [None] * BPC

            def queue_conv2(img):
                state_out[img] = [
                    opool.tile([128, H * W], f32, tag=f"o_{mc}", name=f"o_{mc}")
                    for mc in range(2)
                ]

                def mk(t, mc):
                    def emit():
                        ps = ps2p.tile([128, 512], f32, tag="ps2", name="ps2")
                        nc.tensor.matmul(
                            ps[:, :TW], w1v[:, mc], segs[img][t][:, :, :],
                            start=True, stop=True, perf_mode=DR,
                        )
                        sl = pbs[img][mc][:, TW * t : TW * (t + 1)]
                        # (d): u = psum2*sh2 + p1, in place over p1
                        nc.vector.scalar_tensor_tensor(
                            sl, ps[:, :TW], cc(mc, 7), sl, Alu.mult, Alu.add
                        )
                    return emit

                for t in range(NT):
                    for mc in range(2):
                        pending.append(mk(t, mc))

            def conv1(img):
                tbs[img] = [
                    tpool.tile([128, TOT], bf16, tag=f"t_{mc}", name=f"t_{mc}")
                    for mc in range(2)
                ]
                pbs[img] = [
                    p1pool.tile([128, TOT], bf16, tag=f"p1_{mc}", name=f"p1_{mc}")
                    for mc in range(2)
                ]
                segs[img] = []
                for t in range(NT):
                    seg = a2pool.tile([128, 2, TW], fp8, tag="a2", name="a2")
                    segs[img].append(seg)
                    for mc in range(2):
                        ps = ps1p.tile([128, 512], f32, tag="ps1", name="ps1")
                        for sh in range(9):
                            kh, kw = divmod(sh, 3)
                            nc.tensor.matmul(
                                ps[:, :TW], w3v[:, sh * 2 + mc],
                                a1_rhs(img, t, kh, kw),
                                start=(sh == 0), stop=(sh == 8), perf_mode=DR,
                            )
                        tsl = tbs[img][mc][:, TW * t : TW * (t + 1)]
                        # (a): t = psum1*sh1 + xprep
                        nc.vector.scalar_tensor_tensor(
                            tsl, ps[:, :TW], cc(mc, 6),
                            xprep_slice(img, t, mc), Alu.mult, Alu.add,
                        )
                        if pending:
                            pending.popleft()()
                        # (c): a2' = 2*[t >= tau]  in {0,2} fp8
                        nc.gpsimd.tensor_scalar(
                            seg[:, mc, :], tsl, cc(mc, 1), 2.0,
                            Alu.is_ge, Alu.mult,
                        )
                    # hooks
                    if t == 0 and img < BPC - 1:
                        prep_mono(img + 1)
                        prep_mono_dma(img + 1, 0)
                    elif t == 2 and img < BPC - 1:
                        prep_mono_dma(img + 1, 1)
                    elif t == 3:
                        emit_b(img, 0, 4 * TW)
                    elif t == 4:
                        if img < BPC - 1:
                            prep_mono_sign(img + 1, 0)
                        if img > 0:
                            emit_eg(img - 1, 0, 28)
                    elif t == 5:
                        if img < BPC - 1:
                            prep_mono_sign(img + 1, 1)
                # post-loop
                emit_b(img, 4 * TW, TOT)
                if img > 0:
                    emit_eg(img - 1, 28, 56)

            prep_img0()
            for img in range(BPC):
                conv1(img)
                queue_conv2(img)
            # tail: drain image 3's conv2 with interleaved finalize
            for _ in range(8):
                pending.popleft()()
            emit_eg(BPC - 1, 0, 28)
            while pending:
                pending.popleft()()
            emit_eg(BPC - 1, 28, 56)

    _CACHE[key] = nc
    return nc


def _host_fold(w3, w1, b11, b12, b13, b21, b22, b23,
               g1, be1, m1, v1, g2, be2, m2, v2, a1, a2):
    f = np.float32
    s3 = np.mean(np.abs(w3), axis=(1, 2, 3)).astype(f)
    s1 = np.mean(np.abs(w1), axis=(1, 2, 3)).astype(f)
    inv1 = (g1 / np.sqrt(v1 + EPS)).astype(f)
    inv2 = (g2 / np.sqrt(v2 + EPS)).astype(f)
    sh1 = s3 * inv1
    ch1 = be1 - m1 * inv1
    sh2 = s1 * inv2
    ch2 = be2 - m2 * inv2
    K1 = (ch1 + b12).astype(f)
    bias1 = (b11 - K1).astype(f)
    beta = (b13 + b21).astype(f)
    # a2_true = sign(prelu(t, a1) + beta) = sign(t - tau) for a1 > 0
    tau = np.where(beta >= 0, -beta / a1, -beta).astype(f)
    # conv2 on a2' = a2_true + 1 in {0,2}: psum' = psum_true + rowsum(signW1)
    R = np.sum(np.sign(w1), axis=(1, 2, 3)).astype(f)
    K2p = (ch2 + b13 + b22 - sh2 * R).astype(f)

    fp8 = ml_dtypes.float8_e4m3
    # DoubleRow lhsT layout: [k, ((sh*2+mc)*2+i)*128+m] with i the K-half
    W3 = np.sign(w3).astype(fp8)                                # [O, I, 3, 3]
    W3 = W3.reshape(2, 128, 2, 128, 3, 3)                       # [mc, m, i, k, kh, kw]
    W3 = W3.transpose(3, 4, 5, 0, 2, 1)                         # [k, kh, kw, mc, i, m]
    W3f = np.ascontiguousarray(W3.reshape(128, 9 * 2 * 2 * 128))
    W1 = np.sign(w1).astype(fp8)                                # [O, I, 1, 1]
    W1 = W1.reshape(2, 128, 2, 128)                             # [mc, m, i, k]
    W1 = W1.transpose(3, 0, 2, 1)                               # [k, mc, i, m]
    W1f = np.ascontiguousarray(W1.reshape(128, 2 * 2 * 128))

    consts = np.zeros((2, 128, 8), f)
    for kc in range(2):
        sl = slice(kc * 128, (kc + 1) * 128)
        consts[kc, :, 0] = bias1[sl]
        consts[kc, :, 1] = tau[sl]
        consts[kc, :, 2] = K2p[sl]
        consts[kc, :, 3] = a1[sl]
        consts[kc, :, 4] = a2[sl]
        consts[kc, :, 5] = b23[sl]
        consts[kc, :, 6] = sh1[sl]
        consts[kc, :, 7] = sh2[sl]
    ok = bool((a1 > 0).all()) and bool(np.isfinite(consts).all())
    return W3f, W1f, consts, K1, ok, bool(np.any(b23 != 0))


def _run(in_maps, trace=False, tmpdir=None, trace_kwargs={}):
    from concourse import bass_utils

    nc = build_nc()
    skey = ("split", _FLAGS["has_b23"])
    if not _CACHE.get(skey):
        # walrus workaround applied only for the HW path (CoreSim rejects
        # post-scheduling instruction edits)
        nc.m = _split_drain_waits(nc.m)
        _CACHE[skey] = True
    return bass_utils.run_bass_kernel_spmd(
        nc,
        in_maps,
        core_ids=list(range(NCORES)),
        trace=trace,
        tmpdir=tmpdir,
        trace_kwargs=trace_kwargs,
    )


def make_in_maps(x, w3, w1, **params):
    x = np.asarray(x, np.float32)
    W3f, W1f, consts, K1, ok, has_b23 = _host_fold(
        np.asarray(w3, np.float32), np.asarray(w1, np.float32),
        **{k: np.asarray(v, np.float32) for k, v in params.items()})
    _FLAGS["has_b23"] = has_b23
    _FLAGS["ok"] = ok
    xp = np.zeros((x.shape[0], C, H, PH), np.float32)
    xp[:, :, :, 1:57] = x + K1[None, :, None, None]
    x_prep = xp.reshape(NCORES, BPC, 2, 128, TOT)
    return [
        {"xprep": np.ascontiguousarray(x_prep[c]), "w3f": W3f, "w1f": W1f,
         "consts": consts}
        for c in range(NCORES)
    ]


def assemble_out(results):
    outs = [results[c]["out"].reshape(BPC, C, H, W) for c in range(NCORES)]
    return np.ascontiguousarray(
        np.concatenate(outs, axis=0).astype(np.float32)
    )


def _fallback_numpy(x, w3, w1, b11, b12, b13, b21, b22, b23,
                    g1, be1, m1, v1, g2, be2, m2, v2, a1, a2):
    # Straightforward reference math in numpy; only used if an assumption of
    # the device kernel (a1 > 0, finite folded consts) is violated.
    def cb(p):
        return p[None, :, None, None]

    def conv_np(a, w, pad):
        N, Ci, Hh, Ww = a.shape
        O, I, kh, kw = w.shape
        ap = np.pad(a, ((0, 0), (0, 0), (pad, pad), (pad, pad)))
        out = np.zeros((N, O, Hh, Ww), np.float32)
        wm = w.reshape(O, -1)
        for n in range(N):
            cols = np.empty((I * kh * kw, Hh * Ww), np.float32)
            idx = 0
            for i in range(I):
                for dh in range(kh):
                    for dw in range(kw):
                        cols[idx] = ap[n, i, dh : dh + Hh, dw : dw + Ww].ravel()
                        idx += 1
            out[n] = (wm @ cols).reshape(O, Hh, Ww)
        return out

    def bn(t, g, b, mm, v):
        inv = g / np.sqrt(v + EPS)
        return t * cb(inv) + cb(b - mm * inv)

    def prelu(t, a):
        return np.where(t > 0, t, cb(a) * t)

    s3 = np.mean(np.abs(w3), axis=(1, 2, 3), keepdims=True)
    s1 = np.mean(np.abs(w1), axis=(1, 2, 3), keepdims=True)
    o1 = conv_np(np.sign(x + cb(b11)), np.sign(w3) * s3, 1)
    o1 = x + bn(o1, g1, be1, m1, v1)
    o1 = prelu(o1 + cb(b12), a1) + cb(b13)
    o2 = conv_np(np.sign(o1 + cb(b21)), np.sign(w1) * s1, 0)
    o2 = bn(o2, g2, be2, m2, v2) + o1
    o2 = prelu(o2 + cb(b22), a2) + cb(b23)
    return o2.astype(np.float32)


def kernel(**inputs):
    inputs = {k: np.asarray(v) for k, v in inputs.items()}
    in_maps = make_in_maps(**inputs)
    if not _FLAGS.get("ok", True):
        return _fallback_numpy(**{k: np.asarray(v, np.float32)
                                  for k, v in inputs.items()})
    res = _run(in_maps, trace=False)
    return assemble_out(res.results)


# revision 7
# speedup vs baseline: 2.7815x; 2.7815x over previous
"""Trainium2 Bass kernel for the binary-conv BasicBlock (dense_cnn).

Computation (forward values only):
  A1   = sign(x + b11)
  out1 = x + bn1(conv3x3(A1, binw(w3)))          binw(w) = mean|w| * sign(w)
  o1   = prelu(out1 + b12, a1) + b13
  A2   = sign(o1 + b21)
  out2 = bn2(conv1x1(A2, binw(w1))) + o1
  out  = prelu(out2 + b22, a2) + b23

Strategy: data-parallel over the batch axis, 4 images per NeuronCore on 8
cores; weights/consts replicated.  Per core the 3x3 binary conv runs as 9
shifted DoubleRow fp8 matmuls over 464-column tiles spanning padded rows
1..56 only (3248 cols).  Element-wise work is spread over all three
engines so the TensorE (~27.6us/image) is the only bottleneck (GpSimd's
software tensor ops measure ~10x below roofline, so it only gets memsets):
  DVE:    (a) t = psum1*sh1 + xprep      (d) u = psum2*sh2 + p1   [in-place]
          (b) p1 = max(a1*t, t) in bf16 (2x DVE mode; needs 0 < a1 <= 1)
  ACT:    prep sign(A1), (c) a2 = Sign(t - tau), (e) prelu(u + K2', a2)
a2 = sign(t - tau) replaces sign(prelu(t,a1)+beta) exactly for a1 > 0
(tau = -beta/a1 if beta >= 0 else -beta).  conv2 matmuls of image i are
interleaved into image i+1's conv1 PE stream via a pending queue so the
PE never idles on drains.  Image 0's A1 is built in 4 row-bands so the
first matmul starts ~3us in.
"""

import numpy as np
import ml_dtypes
from collections import deque

C = 256
H = W = 56
PH = 58                    # padded image side
NPIX = PH * PH             # 3364
TW = 464                   # matmul tile width (8 padded rows)
NT = 7                     # tiles per image
TOT = NT * TW              # 3248 = 56 rows * 58 cols  (padded rows 1..56)
G0 = PH                    # first computed pixel (row 1, col 0) in A1 coords
HALO = 16
A1W = 3408                 # 16 + NPIX + 16 -> next multiple of 16
BPC = 4                    # images per core
NCORES = 8
EPS = 1e-5

# img0 band layout: (first padded row, end padded row, MM tiles in band)
BANDS = [(0, 19, (0, 1)), (16, 35, (2, 3)), (32, 51, (4, 5)), (48, 58, (6,))]
TILE2BAND = [0, 0, 1, 1, 2, 2, 3]


def _pad16(w):
    return (w + 15) // 16 * 16


_CACHE = {}
_FLAGS = {"has_b23": False}


def _split_drain_waits(m, max_waits=1):
    """This toolchain's walrus rejects instructions carrying more than ~1-2
    sync waits; hoist extra waits onto preceding single-wait EventSemaphore
    ops on the same engine (semantically identical: the engine blocks on
    each wait in sequence before executing the instruction)."""
    import copy as _copy
    from concourse import mybir

    new_module = _copy.replace(m, functions=[])
    for function in m.functions:
        new_function = _copy.replace(function, blocks=[])
        new_function.set_allocations_from_list(function.allocations)
        for block in function.blocks:
            out = []
            for inst in block.instructions:
                si = inst.sync_info
                if si is not None and len(si.on_wait) > max_waits:
                    waits = list(si.on_wait)
                    keep = waits[:max_waits] if not isinstance(
                        inst, mybir.InstDrain) else []
                    hoist = waits[len(keep):]
                    for i, wt in enumerate(hoist):
                        out.append(
                            mybir.InstEventSemaphore(
                                name=f"{inst.name}-wsplit{i}",
                                opcode="EventSemaphore",
                                engine=inst.engine,
                                sync_info=mybir.SyncInfo(on_wait=[wt], on_update=[]),
                            )
                        )
                    inst.sync_info = mybir.SyncInfo(
                        on_wait=keep, on_update=list(si.on_update)
                    )
                out.append(inst)
            new_block = _copy.replace(block, instructions=out)
            new_function.blocks.append(new_block)
        new_module.functions.append(new_function)
    return new_module


def build_nc(has_b23=None):
    """Build (once per structure flag) the per-core Bass program."""
    if has_b23 is None:
        has_b23 = _FLAGS["has_b23"]
    key = ("nc", has_b23)
    if key in _CACHE:
        return _CACHE[key]
    import concourse.bass as bass
    import concourse.tile as tile
    from concourse import mybir

    Alu = mybir.AluOpType
    AF = mybir.ActivationFunctionType
    f32 = mybir.dt.float32
    bf16 = mybir.dt.bfloat16
    fp8 = mybir.dt.float8e4
    DR = mybir.MatmulPerfMode.DoubleRow

    nc = bass.Bass(trn_type="TRN2", debug=False)
    # xprep = x + K1, stored for padded rows 1..56 (58 cols each, col 0/57
    # are zero pads): dram index = (row-1)*58 + col
    x_d = nc.dram_tensor("xprep", [BPC, 2, 128, TOT], f32, kind="ExternalInput")
    w3_d = nc.dram_tensor("w3f", [128, 9 * 2 * 2 * 128], fp8, kind="ExternalInput")
    w1_d = nc.dram_tensor("w1f", [128, 2 * 2 * 128], fp8, kind="ExternalInput")
    c_d = nc.dram_tensor("consts", [2, 128, 8], f32, kind="ExternalInput")
    cb_d = nc.dram_tensor("constsb", [2, 128, 8], bf16, kind="ExternalInput")
    o_d = nc.dram_tensor("out", [BPC, 2, 128, H * W], f32, kind="ExternalOutput")

    with tile.TileContext(nc) as tc:
        with (
            tc.tile_pool(name="wpool", bufs=1) as wpool,
            tc.tile_pool(name="xbpool", bufs=1) as xbpool,
            tc.tile_pool(name="a1bpool", bufs=1) as a1bpool,
            tc.tile_pool(name="xpool", bufs=2) as xpool,
            tc.tile_pool(name="a1pool", bufs=2) as a1pool,
            tc.tile_pool(name="tpool", bufs=2) as tpool,
            tc.tile_pool(name="p1pool", bufs=2) as p1pool,
            tc.tile_pool(name="a2pool", bufs=2) as a2pool,
            tc.tile_pool(name="opool", bufs=1) as opool,
            tc.tile_pool(name="ps1", bufs=4, space="PSUM") as ps1p,
            tc.tile_pool(name="ps2", bufs=4, space="PSUM") as ps2p,
        ):
            # ---- constants / weights (resident) ----
            w3sb = wpool.tile([128, 9 * 2 * 2 * 128], fp8, tag="w3")
            nc.sync.dma_start(w3sb[:], w3_d.ap())
            w1sb = wpool.tile([128, 2 * 2 * 128], fp8, tag="w1")
            nc.sync.dma_start(w1sb[:], w1_d.ap())
            w3v = w3sb[:].rearrange("p (g two m) -> p g two m", two=2, m=128)
            w1v = w1sb[:].rearrange("p (g two m) -> p g two m", two=2, m=128)
            csb, csbb = [], []
            for kc in range(2):
                ct = wpool.tile([128, 8], f32, tag=f"c_{kc}")
                nc.sync.dma_start(ct[:], c_d.ap()[kc])
                csb.append(ct)
                cbt = wpool.tile([128, 8], bf16, tag=f"cb_{kc}")
                nc.sync.dma_start(cbt[:], cb_d.ap()[kc])
                csbb.append(cbt)

            def cc(kc, j):
                return csb[kc][:, j : j + 1]

            def ccb(kc, j):
                return csbb[kc][:, j : j + 1]

            # const j-layout: 0 bias1 (prep sign), 1 tau, 2 K2', 3 a1,
            # 4 a2, 5 b23, 6 sh1, 7 sh2

            # ---- per-image state ----
            xb = {}    # (band, kc) -> img0 xprep band tile
            a1b = []   # img0 A1 band tiles
            xm = [None] * BPC   # mono xprep tiles (imgs 1..3): [kc]
            a1m = [None] * BPC  # mono A1 tiles (imgs 1..3)
            tbs = [None] * BPC  # t tiles per mc (bf16)
            pbs = [None] * BPC  # p1/u tiles per mc (bf16)
            a2s = [None] * BPC  # a2 tiles [128, 2, TOT] fp8
            pending = deque()   # conv2 closures: one MM2 + one (d) each

            def memset_borders(a1t, r0, r1, width):
                # zero every A1 element a matmul may read that sign won't
                # write: halo, row 0 / row 57, and col 0/57 of each row.
                nrow = r1 - r0
                for kc in range(2):
                    v = a1t[:].rearrange("p (two w) -> p two w", two=2)[:, kc]
                    head = HALO + (PH + 1 if r0 == 0 else 1)
                    nc.gpsimd.memset(v[:, 0:head], 0.0)
                    # (row r, c57)+(row r+1, c0) pairs for r in [r0, r1-1)
                    pairs = v[
                        :, HALO + 57 : HALO + 57 + (nrow - 1) * PH
                    ].rearrange("p (h w) -> p h w", w=PH)[:, :, 0:2]
                    nc.gpsimd.memset(pairs, 0.0)
                    if r1 == PH:
                        # row 57 (minus its c0, already in the last pair)
                        # plus right halo / alignment tail
                        nc.gpsimd.memset(
                            v[:, HALO + (57 - r0) * PH + 1 : width], 0.0
                        )
                    else:
                        # last row's c57 (+1 spare into the unread margin)
                        nc.gpsimd.memset(
                            v[:, HALO + (nrow - 1) * PH + 57 :
                              HALO + (nrow - 1) * PH + 59], 0.0
                        )

            def prep_img0():
                for b, (r0, r1, _tiles) in enumerate(BANDS):
                    vr0, vr1 = max(r0, 1), min(r1, 57)
                    for kc in range(2):
                        xt = xbpool.tile(
                            [128, (vr1 - vr0) * PH], f32, tag=f"xb{b}_{kc}",
                            name=f"xb{b}_{kc}",
                        )
                        nc.sync.dma_start(
                            xt[:],
                            x_d.ap()[0, kc][:, (vr0 - 1) * PH : (vr1 - 1) * PH],
                        )
                        xb[(b, kc)] = xt
                for b, (r0, r1, _tiles) in enumerate(BANDS):
                    width = _pad16(HALO + (r1 - r0) * PH + HALO)
                    a1t = a1bpool.tile([128, 2 * width], fp8, tag=f"a1b{b}",
                                       name=f"a1b{b}")
                    a1b.append(a1t)
                    memset_borders(a1t, r0, r1, width)
                    vr0, vr1 = max(r0, 1), min(r1, 57)
                    nvr = vr1 - vr0
                    for kc in range(2):
                        dst = a1t[:].rearrange("p (two w) -> p two w", two=2)[
                            :, kc, HALO + (vr0 - r0) * PH : HALO + (vr1 - r0) * PH
                        ].rearrange("p (h w) -> p h w", w=PH)[:, :, 1:57]
                        src = xb[(b, kc)][:].rearrange(
                            "p (h w) -> p h w", w=PH
                        )[:, :, 1:57]
                        nc.scalar.activation(dst, src, AF.Sign, bias=cc(kc, 0))

            def prep_mono(img):
                # called from hooks of conv1(img-1): t0/t2 DMA, t4/t5 sign
                xm[img] = [
                    xpool.tile([128, TOT], f32, tag=f"xk_{kc}", name=f"xk_{kc}")
                    for kc in range(2)
                ]
                a1m[img] = a1pool.tile([128, 2 * A1W], fp8, tag="a1m", name="a1m")

            def prep_mono_dma(img, kc):
                nc.sync.dma_start(xm[img][kc][:], x_d.ap()[img, kc])

            def prep_mono_sign(img, kc):
                if kc == 0:
                    memset_borders(a1m[img], 0, PH, A1W)
                dst = a1m[img][:].rearrange("p (two w) -> p two w", two=2)[
                    :, kc, HALO + G0 : HALO + G0 + TOT
                ].rearrange("p (h w) -> p h w", w=PH)[:, :, 1:57]
                src = xm[img][kc][:].rearrange("p (h w) -> p h w", w=PH)[:, :, 1:57]
                nc.scalar.activation(dst, src, AF.Sign, bias=cc(kc, 0))

            def a1_rhs(img, t, kh, kw):
                if img == 0:
                    b = TILE2BAND[t]
                    r0 = BANDS[b][0]
                    base = HALO + (G0 + TW * t - r0 * PH)
                    v = a1b[b][:].rearrange("p (two w) -> p two w", two=2)
                else:
                    base = HALO + G0 + TW * t
                    v = a1m[img][:].rearrange("p (two w) -> p two w", two=2)
                off = base + (kh - 1) * PH + (kw - 1)
                return v[:, :, off : off + TW]

            def xprep_slice(img, t, mc):
                if img == 0:
                    b = TILE2BAND[t]
                    vr0 = max(BANDS[b][0], 1)
                    lo = TW * t - (vr0 - 1) * PH
                    return xb[(b, mc)][:, lo : lo + TW]
                return xm[img][mc][:, TW * t : TW * (t + 1)]

            def emit_b(img, lo, hi):
                # p1 = max(a1*t, t) on DVE, all-bf16 for the 2x mode
                for mc in range(2):
                    nc.vector.scalar_tensor_tensor(
                        pbs[img][mc][:, lo:hi], tbs[img][mc][:, lo:hi],
                        ccb(mc, 3), tbs[img][mc][:, lo:hi],
                        Alu.mult, Alu.max,
                    )

            def emit_c(img, lo, hi):
                # a2 = Sign(t - tau) on ACT (bias slot carries -tau)
                for mc in range(2):
                    nc.scalar.activation(
                        a2s[img][:, mc, lo:hi], tbs[img][mc][:, lo:hi],
                        AF.Sign, bias=cc(mc, 1),
                    )

            def emit_eg(img, h0, h1):
                # (e) prelu(u + K2', a2) interior -> compact out, (g) +b23,
                # then DMA the finished rows
                for mc in range(2):
                    ot = state_out[img][mc]
                    dst = ot[:].rearrange("p (h w) -> p h w", w=W)[:, h0:h1, :]
                    src = pbs[img][mc][:].rearrange(
                        "p (h w) -> p h w", w=PH
                    )[:, h0:h1, 1:57]
                    nc.scalar.activation(
                        dst, src, AF.Prelu, bias=cc(mc, 2), alpha=cc(mc, 4)
                    )
                    if has_b23:
                        nc.gpsimd.tensor_scalar(
                            dst, dst, cc(mc, 5), None, Alu.add
                        )
                    nc.sync.dma_start(
                        o_d.ap()[img, mc][:, h0 * W : h1 * W],
                        ot[:, h0 * W : h1 * W],
                    )

            state_out = [None] * BPC

            def queue_conv2(img):
                state_out[img] = [
                    opool.tile([128, H * W], f32, tag=f"o_{mc}", name=f"o_{mc}")
                    for mc in range(2)
                ]

                def mk(t, mc):
                    def emit():
                        ps = ps2p.tile([128, 512], f32, tag="ps2", name="ps2")
                        nc.tensor.matmul(
                            ps[:, :TW], w1v[:, mc],
                            a2s[img][:, :, TW * t : TW * (t + 1)],
                            start=True, stop=True, perf_mode=DR,
                        )
                        sl = pbs[img][mc][:, TW * t : TW * (t + 1)]
                        # (d): u = psum2*sh2 + p1, in place over p1
                        nc.vector.scalar_tensor_tensor(
                            sl, ps[:, :TW], cc(mc, 7), sl, Alu.mult, Alu.add
                        )
                    return emit

                for t in range(NT):
                    for mc in range(2):
                        pending.append(mk(t, mc))

            def conv1(img):
                tbs[img] = [
                    tpool.tile([128, TOT], bf16, tag=f"t_{mc}", name=f"t_{mc}")
                    for mc in range(2)
                ]
                pbs[img] = [
                    p1pool.tile([128, TOT], bf16, tag=f"p1_{mc}", name=f"p1_{mc}")
                    for mc in range(2)
                ]
                a2s[img] = a2pool.tile([128, 2, TOT], fp8, tag="a2", name="a2")
                for t in range(NT):
                    for mc in range(2):
                        ps = ps1p.tile([128, 512], f32, tag="ps1", name="ps1")
                        for sh in range(9):
                            kh, kw = divmod(sh, 3)
                            nc.tensor.matmul(
                                ps[:, :TW], w3v[:, sh * 2 + mc],
                                a1_rhs(img, t, kh, kw),
                                start=(sh == 0), stop=(sh == 8), perf_mode=DR,
                            )
                        tsl = tbs[img][mc][:, TW * t : TW * (t + 1)]
                        # (a): t = psum1*sh1 + xprep
                        nc.vector.scalar_tensor_tensor(
                            tsl, ps[:, :TW], cc(mc, 6),
                            xprep_slice(img, t, mc), Alu.mult, Alu.add,
                        )
                        if pending:
                            pending.popleft()()
                    # hooks
                    if t == 0 and img < BPC - 1:
                        prep_mono(img + 1)
                        prep_mono_dma(img + 1, 0)
                    elif t == 2 and img < BPC - 1:
                        prep_mono_dma(img + 1, 1)
                    elif t == 3:
                        emit_b(img, 0, 4 * TW)
                        emit_c(img, 0, 4 * TW)
                    elif t == 4:
                        if img < BPC - 1:
                            prep_mono_sign(img + 1, 0)
                        if img > 0:
                            emit_eg(img - 1, 0, 28)
                    elif t == 5:
                        if img < BPC - 1:
                            prep_mono_sign(img + 1, 1)
                # post-loop
                emit_b(img, 4 * TW, TOT)
                emit_c(img, 4 * TW, TOT)
                if img > 0:
                    emit_eg(img - 1, 28, 56)

            prep_img0()
            for img in range(BPC):
                conv1(img)
                queue_conv2(img)
            # tail: drain image 3's conv2 with interleaved finalize
            for _ in range(8):
                pending.popleft()()
            emit_eg(BPC - 1, 0, 28)
            while pending:
                pending.popleft()()
            emit_eg(BPC - 1, 28, 56)

    _CACHE[key] = nc
    return nc


def _host_fold(w3, w1, b11, b12, b13, b21, b22, b23,
               g1, be1, m1, v1, g2, be2, m2, v2, a1, a2):
    f = np.float32
    s3 = np.mean(np.abs(w3), axis=(1, 2, 3)).astype(f)
    s1 = np.mean(np.abs(w1), axis=(1, 2, 3)).astype(f)
    inv1 = (g1 / np.sqrt(v1 + EPS)).astype(f)
    inv2 = (g2 / np.sqrt(v2 + EPS)).astype(f)
    sh1 = s3 * inv1
    ch1 = be1 - m1 * inv1
    sh2 = s1 * inv2
    ch2 = be2 - m2 * inv2
    K1 = (ch1 + b12).astype(f)
    bias1 = (b11 - K1).astype(f)
    beta = (b13 + b21).astype(f)
    # a2 = sign(prelu(t, a1) + beta) = sign(t - tau) for a1 > 0
    tau = np.where(beta >= 0, -beta / a1, -beta).astype(f)
    K2p = (ch2 + b13 + b22).astype(f)

    fp8 = ml_dtypes.float8_e4m3
    # DoubleRow lhsT layout: [k, ((sh*2+mc)*2+i)*128+m] with i the K-half
    W3 = np.sign(w3).astype(fp8)                                # [O, I, 3, 3]
    W3 = W3.reshape(2, 128, 2, 128, 3, 3)                       # [mc, m, i, k, kh, kw]
    W3 = W3.transpose(3, 4, 5, 0, 2, 1)                         # [k, kh, kw, mc, i, m]
    W3f = np.ascontiguousarray(W3.reshape(128, 9 * 2 * 2 * 128))
    W1 = np.sign(w1).astype(fp8)                                # [O, I, 1, 1]
    W1 = W1.reshape(2, 128, 2, 128)                             # [mc, m, i, k]
    W1 = W1.transpose(3, 0, 2, 1)                               # [k, mc, i, m]
    W1f = np.ascontiguousarray(W1.reshape(128, 2 * 2 * 128))

    consts = np.zeros((2, 128, 8), f)
    for kc in range(2):
        sl = slice(kc * 128, (kc + 1) * 128)
        consts[kc, :, 0] = bias1[sl]
        consts[kc, :, 1] = -tau[sl]
        consts[kc, :, 2] = K2p[sl]
        consts[kc, :, 3] = a1[sl]
        consts[kc, :, 4] = a2[sl]
        consts[kc, :, 5] = b23[sl]
        consts[kc, :, 6] = sh1[sl]
        consts[kc, :, 7] = sh2[sl]
    ok = (bool((a1 > 0).all()) and bool((a1 <= 1).all())
          and bool(np.isfinite(consts).all()))
    return W3f, W1f, consts, K1, ok, bool(np.any(b23 != 0))


def _run(in_maps, trace=False, tmpdir=None, trace_kwargs={}):
    from concourse import bass_utils

    nc = build_nc()
    skey = ("split", _FLAGS["has_b23"])
    if not _CACHE.get(skey):
        # walrus workaround applied only for the HW path (CoreSim rejects
        # post-scheduling instruction edits)
        nc.m = _split_drain_waits(nc.m)
        _CACHE[skey] = True
    return bass_utils.run_bass_kernel_spmd(
        nc,
        in_maps,
        core_ids=list(range(NCORES)),
        trace=trace,
        tmpdir=tmpdir,
        trace_kwargs=trace_kwargs,
    )


def make_in_maps(x, w3, w1, **params):
    x = np.asarray(x, np.float32)
    W3f, W1f, consts, K1, ok, has_b23 = _host_fold(
        np.asarray(w3, np.float32), np.asarray(w1, np.float32),
        **{k: np.asarray(v, np.float32) for k, v in params.items()})
    _FLAGS["has_b23"] = has_b23
    _FLAGS["ok"] = ok
    xp = np.zeros((x.shape[0], C, H, PH), np.float32)
    xp[:, :, :, 1:57] = x + K1[None, :, None, None]
    x_prep = xp.reshape(NCORES, BPC, 2, 128, TOT)
    constsb = consts.astype(ml_dtypes.bfloat16)
    return [
        {"xprep": np.ascontiguousarray(x_prep[c]), "w3f": W3f, "w1f": W1f,
         "consts": consts, "constsb": constsb}
        for c in range(NCORES)
    ]


def assemble_out(results):
    outs = [results[c]["out"].reshape(BPC, C, H, W) for c in range(NCORES)]
    return np.ascontiguousarray(
        np.concatenate(outs, axis=0).astype(np.float32)
    )


def _fallback_numpy(x, w3, w1, b11, b12, b13, b21, b22, b23,
                    g1, be1, m1, v1, g2, be2, m2, v2, a1, a2):
    # Straightforward reference math in numpy; only used if an assumption of
    # the device kernel (a1 > 0, finite folded consts) is violated.
    def cb(p):
        return p[None, :, None, None]

    def conv_np(a, w, pad):
        N, Ci, Hh, Ww = a.shape
        O, I, kh, kw = w.shape
        ap = np.pad(a, ((0, 0), (0, 0), (pad, pad), (pad, pad)))
        out = np.zeros((N, O, Hh, Ww), np.float32)
        wm = w.reshape(O, -1)
        for n in range(N):
            cols = np.empty((I * kh * kw, Hh * Ww), np.float32)
            idx = 0
            for i in range(I):
                for dh in range(kh):
                    for dw in range(kw):
                        cols[idx] = ap[n, i, dh : dh + Hh, dw : dw + Ww].ravel()
                        idx += 1
            out[n] = (wm @ cols).reshape(O, Hh, Ww)
        return out

    def bn(t, g, b, mm, v):
        inv = g / np.sqrt(v + EPS)
        return t * cb(inv) + cb(b - mm * inv)

    def prelu(t, a):
        return np.where(t > 0, t, cb(a) * t)

    s3 = np.mean(np.abs(w3), axis=(1, 2, 3), keepdims=True)
    s1 = np.mean(np.abs(w1), axis=(1, 2, 3), keepdims=True)
    o1 = conv_np(np.sign(x + cb(b11)), np.sign(w3) * s3, 1)
    o1 = x + bn(o1, g1, be1, m1, v1)
    o1 = prelu(o1 + cb(b12), a1) + cb(b13)
    o2 = conv_np(np.sign(o1 + cb(b21)), np.sign(w1) * s1, 0)
    o2 = bn(o2, g2, be2, m2, v2) + o1
    o2 = prelu(o2 + cb(b22), a2) + cb(b23)
    return o2.astype(np.float32)


def kernel(**inputs):
    inputs = {k: np.asarray(v) for k, v in inputs.items()}
    in_maps = make_in_maps(**inputs)
    if not _FLAGS.get("ok", True):
        return _fallback_numpy(**{k: np.asarray(v, np.float32)
                                  for k, v in inputs.items()})
    res = _run(in_maps, trace=False)
    return assemble_out(res.results)
